# revision 30
# baseline (speedup 1.0000x reference)
"""Trainium2 Bass kernel for nn_NormConvTranspose2d.

Math: a stride-2, pad-1, k=4 depthwise ConvTranspose2d splits into 4 output
parity classes (py, px), each a 2x2-tap conv over a 48x48 sub-image.  The
normalizer (convT of ones) is piecewise constant per (cin, cout, class,
border-region), so its reciprocal folds into the conv weights on the host,
turning sum_cin(num/den) into plain matmuls over Cin.  Border rows/cols get
additive inclusion-exclusion delta matmuls.  The per-channel bias cancels
under InstanceNorm and is dropped.

Sharding: batch (2) x Cout-groups (4x16) across the 8 cores.  Each core owns
one batch and 16 output channels with all 4 parity classes, so InstanceNorm
is fully core-local.  Core layout: two M=32 blocks (px=0 at psum partitions
0:32, px=1 at 32:64; rows = 16*py + i), streamed through two PE column
groups.
"""

import functools

import numpy as np

B = 2
CIN = 64
COUT = 64
S = 48            # class sub-image side
NCORES = 8
CSH = 16          # couts per core
EPS = 1e-10
IN_EPS = 1e-5
NCH = 6           # spatial row-chunks per class
CHR = S // NCH    # rows per chunk (8)
NPB = CHR * S     # matmul free size per chunk (384)


def _ky(p, a):
    # kernel tap index along one dim for parity p, tap a (a=0: dy=0 tap)
    if p == 0:
        return 1 if a == 0 else 3
    return 2 if a == 0 else 0


def _host_weights(W, g):
    """Folded weight slabs for cout group g (16 channels). M=32: 16*py+i."""
    s = slice(CSH * g, CSH * (g + 1))
    wmain = np.zeros((128, 6, 32), np.float32)
    wrow = np.zeros((128, 4, 32), np.float32)
    wcol = np.zeros((64, 6, 32), np.float32)
    wcor = np.zeros((64, 4, 32), np.float32)
    for px in range(2):
        for py in range(2):
            Wt = {(a, b): W[:, s, _ky(py, a), _ky(px, b)].astype(np.float32)
                  for a in range(2) for b in range(2)}
            rI = 1.0 / (Wt[0, 0] + Wt[0, 1] + Wt[1, 0] + Wt[1, 1] + EPS)
            rR = 1.0 / (Wt[0, 0] + Wt[0, 1] + EPS)
            rC = 1.0 / (Wt[0, 0] + Wt[1, 0] + EPS)
            rX = 1.0 / (Wt[0, 0] + EPS)
            co = 16 * py
            # main: m=0 -> dy=0 taps (both py), m=1 -> py0 dy=-1, m=2 -> py1 dy=+1
            wmain[0:64, px * 3 + 0, co:co + 16] = Wt[0, 0] * rI
            wmain[64:128, px * 3 + 0, co:co + 16] = Wt[0, 1] * rI
            m = px * 3 + 1 + py
            wmain[0:64, m, co:co + 16] = Wt[1, 0] * rI
            wmain[64:128, m, co:co + 16] = Wt[1, 1] * rI
            # row-border delta (u = ub), taps a=0 only (K=128 paired)
            wrow[0:64, px * 2 + py, co:co + 16] = Wt[0, 0] * (rR - rI)
            wrow[64:128, px * 2 + py, co:co + 16] = Wt[0, 1] * (rR - rI)
            # col-border delta (v = vb), taps b=0 only (K=64)
            wcol[:, px * 3 + 0, co:co + 16] = Wt[0, 0] * (rC - rI)
            wcol[:, px * 3 + 1 + py, co:co + 16] = Wt[1, 0] * (rC - rI)
            # corner delta
            wcor[:, px * 2 + py, co:co + 16] = Wt[0, 0] * (rX - rR - rC + rI)
    return wmain, wrow, wcol, wcor


def _host_sel():
    # stats replicated to every block position: sel[p, q] = 1/4 iff same cout
    p = np.arange(64)
    sel = 0.25 * (p[:, None] % CSH == p[None, :] % CSH)
    return sel.astype(np.float32)


@functools.lru_cache(maxsize=2)
def _build_nc():
    import concourse.bass as bass
    import concourse.tile as tile
    from concourse import bacc, mybir

    f32 = mybir.dt.float32
    f32r = mybir.dt.float32r
    add = mybir.AluOpType.add
    mult = mybir.AluOpType.mult
    sub = mybir.AluOpType.subtract
    AF = mybir.ActivationFunctionType

    nc = bacc.Bacc("TRN2", target_bir_lowering=False, debug=False)

    xp_t = nc.dram_tensor("xp", [64, 2500], f32r, kind="ExternalInput")
    wmain_t = nc.dram_tensor("wmain", [128, 192], f32r, kind="ExternalInput")
    wrow_t = nc.dram_tensor("wrow", [128, 128], f32, kind="ExternalInput")
    wcol_t = nc.dram_tensor("wcol", [64, 192], f32, kind="ExternalInput")
    wcor_t = nc.dram_tensor("wcor", [64, 128], f32, kind="ExternalInput")
    sel_t = nc.dram_tensor("sel", [64, 64], f32, kind="ExternalInput")
    out_t = nc.dram_tensor("out", [2, 32, NCH, NPB], f32,
                           kind="ExternalOutput")

    with tile.TileContext(nc) as tc:
        with (
            tc.tile_pool(name="xt", bufs=1) as xpool,
            tc.tile_pool(name="wt", bufs=1) as wpool,
            tc.tile_pool(name="sm", bufs=1) as spool,
            tc.tile_pool(name="fin", bufs=1) as fpool,
            tc.tile_pool(name="ps", bufs=1, space="PSUM") as pspool,
            tc.tile_pool(name="pss", bufs=1, space="PSUM") as psmall,
        ):
            # ---- weights first (tiny), then x on two DMA queues ------
            wmain_sb = wpool.tile([128, 6, 32], f32r, name="wm", tag="wm")
            wrow_sb = wpool.tile([128, 4, 32], f32, name="wr", tag="wr")
            wcol_sb = wpool.tile([64, 6, 32], f32, name="wc", tag="wc")
            wcor_sb = wpool.tile([64, 4, 32], f32, name="wx", tag="wx")
            sel_sb = wpool.tile([64, 64], f32, name="sl", tag="sl")
            # lower halves from HBM; upper (col-shifted) built on DVE.
            # pad cols of the shifted copy come from x_pad's zero columns.
            T = [xpool.tile([128, 50, 50], f32r, name=f"t{t}", tag=f"t{t}")
                 for t in range(2)]
            xv = xp_t[:, :].rearrange("p (r s) -> p r s", s=50)
            nc.sync.dma_start(wcol_sb[:, :, :], wcol_t[:, :])
            nc.gpsimd.dma_start(wcor_sb[:, :, :], wcor_t[:, :])
            nc.sync.dma_start(T[0][0:64, :, :], xv[:, :, :])
            nc.scalar.dma_start(T[1][0:64, :, :], xv[:, :, :])
            nc.sync.dma_start(wmain_sb[:, :, :], wmain_t[:, :])
            nc.gpsimd.dma_start(wrow_sb[:, :, :], wrow_t[:, :])
            nc.gpsimd.dma_start(sel_sb[:, :], sel_t[:, :])
            nc.vector.tensor_copy(T[0][64:128, :, 1:50], T[0][0:64, :, 0:49])
            nc.vector.tensor_copy(T[0][64:128, :, 0:1], T[0][0:64, :, 49:50])
            nc.vector.tensor_copy(T[1][64:128, :, 0:49], T[1][0:64, :, 1:50])
            nc.vector.tensor_copy(T[1][64:128, :, 49:50], T[1][0:64, :, 0:1])

            prenorm = fpool.tile([64, NCH, NPB], f32, name="pn", tag="pn")
            DY = (0, -1, +1)

            # ---- col+corner deltas into pc (emitted mid-stream) ------
            pcs_sb = spool.tile([64, 48], f32, name="pcs", tag="pcs")

            def emit_pc():
                for px in range(2):
                    vb = 0 if px == 0 else S - 1
                    Tt = T[px]
                    pc = psmall.tile([32, 48], f32, name=f"pc{px}",
                                     tag=f"pc{px}")
                    for j, dy in enumerate(DY):
                        nc.tensor.matmul(
                            pc[:, 0:48],
                            wcol_sb[:, px * 3 + j, :],
                            Tt[0:64, 1 + dy:49 + dy, 1 + vb:2 + vb].bitcast(f32),
                            start=(j == 0), stop=False,
                            skip_group_check=True,
                        )
                    nc.tensor.matmul(
                        pc[:, 0:1], wcor_sb[:, px * 2 + 0, :],
                        Tt[0:64, 1:2, 1 + vb:2 + vb].bitcast(f32),
                        start=False, stop=False, skip_group_check=True,
                    )
                    nc.tensor.matmul(
                        pc[:, 47:48], wcor_sb[:, px * 2 + 1, :],
                        Tt[0:64, 48:49, 1 + vb:2 + vb].bitcast(f32),
                        start=False, stop=True, skip_group_check=True,
                    )
                    nc.vector.tensor_copy(pcs_sb[32 * px:32 * px + 32, :],
                                          pc[:, :])

            # ---- main matmuls + row deltas, chunk-major; stats pipelined
            stats6 = spool.tile([64, NCH, 6], f32, name="st6", tag="st6")

            def chunk_block(c, px):
                P0 = 32 * px
                vb = 0 if px == 0 else S - 1
                mc = pspool.tile([32, 512], f32, name=f"mc{c}{px}", tag="mc",
                                 bufs=4)
                for m in range(3):
                    dy = DY[m]
                    r0 = CHR * c + 1 + dy
                    nc.tensor.matmul(
                        mc[:, 0:NPB],
                        wmain_sb[:, px * 3 + m, :],
                        T[px][0:128, r0:r0 + CHR, 1:49],
                        start=(m == 0), stop=(m == 2),
                        skip_group_check=True,
                    )
                if c == 0:
                    nc.tensor.matmul(
                        mc[:, 0:48],
                        wrow_sb[:, px * 2 + 0, :],
                        T[px][0:128, 1:2, 1:49].bitcast(f32),
                        start=False, stop=False, skip_group_check=True,
                    )
                if c == NCH - 1:
                    nc.tensor.matmul(
                        mc[:, NPB - 48:NPB],
                        wrow_sb[:, px * 2 + 1, :],
                        T[px][0:128, 48:49, 1:49].bitcast(f32),
                        start=False, stop=False, skip_group_check=True,
                    )
                # copy out to SBUF (ACT for px0, DVE for px1)
                dstc = prenorm[P0:P0 + 32, c, :]
                if px == 0:
                    nc.scalar.activation(dstc, mc[:, 0:NPB], AF.Identity)
                else:
                    nc.vector.tensor_copy(dstc, mc[:, 0:NPB])
                # col-delta add on SBUF (strided RMW; no psum involvement)
                dst = prenorm[P0:P0 + 32, c, :].rearrange(
                    "p (a b) -> p a b", b=S)[:, :, vb:vb + 1].squeeze(2)
                nc.vector.tensor_tensor(
                    dst, dst,
                    pcs_sb[P0:P0 + 32, CHR * c:CHR * (c + 1)], add)

            emit_pc()
            for c in range(NCH):
                chunk_block(c, 0)
                chunk_block(c, 1)
                nc.vector.bn_stats(stats6[:, c, :], prenorm[:, c, :])

            # ---- stats merge + norm coefficients ---------------------
            aggr = spool.tile([64, 2], f32, name="agg", tag="agg")
            nc.vector.bn_aggr(aggr[:, :], stats6[:, :, :])
            rhs2 = spool.tile([64, 2], f32, name="rh2", tag="rh2")
            nc.vector.tensor_copy(rhs2[:, 0:1], aggr[:, 0:1])
            nc.vector.scalar_tensor_tensor(
                rhs2[:, 1:2], in0=aggr[:, 0:1], scalar=aggr[:, 0:1],
                in1=aggr[:, 1:2], op0=mult, op1=add)
            stat_ps = psmall.tile([64, 2], f32, name="stp", tag="pc0")
            nc.tensor.matmul(stat_ps[:, :], sel_sb[:, :], rhs2[:, :],
                             start=True, stop=True)
            sts = spool.tile([64, 2], f32, name="sts", tag="sts")
            nc.vector.tensor_copy(sts[:, :], stat_ps[:, :])
            var16 = spool.tile([64, 1], f32, name="v16", tag="v16")
            nc.vector.scalar_tensor_tensor(
                var16[:, :], in0=sts[:, 0:1], scalar=sts[:, 0:1],
                in1=sts[:, 1:2], op0=mult, op1=sub)
            nc.vector.tensor_scalar_mul(var16[:, :], var16[:, :], -1.0)
            std16 = spool.tile([64, 1], f32, name="s16", tag="s16")
            epst = spool.tile([64, 1], f32, name="eps", tag="eps")
            nc.vector.memset(epst[:, :], IN_EPS)
            nc.scalar.activation(std16[:, :], var16[:, :], AF.Sqrt,
                                 bias=epst[:, :])
            mr = spool.tile([64, 2], f32, name="mr", tag="mr")
            nc.vector.reciprocal(mr[:, 1:2], std16[:, :])
            nc.vector.tensor_tensor(mr[:, 0:1], sts[:, 0:1], mr[:, 1:2], mult)
            nc.vector.tensor_scalar_mul(mr[:, 0:1], mr[:, 0:1], -1.0)

            # ---- normalize + store (ACT px0 | DVE px1, pipelined DMA)
            final = fpool.tile([64, NCH, NPB], f32, name="fin", tag="fin")
            for h in range(2):
                cs, ce = (0, 3) if h == 0 else (3, NCH)
                nc.scalar.activation(final[0:32, cs:ce, :],
                                     prenorm[0:32, cs:ce, :],
                                     AF.Identity,
                                     bias=mr[0:32, 0:1], scale=mr[0:32, 1:2])
                nc.sync.dma_start(out_t[0, :, cs:ce, :], final[0:32, cs:ce, :])
                nc.vector.tensor_scalar(final[32:64, cs:ce, :],
                                        prenorm[32:64, cs:ce, :],
                                        mr[32:64, 1:2], mr[32:64, 0:1],
                                        op0=mult, op1=add)
                nc.gpsimd.dma_start(out_t[1, :, cs:ce, :],
                                    final[32:64, cs:ce, :])

    nc.compile()
    return nc


def _host_inputs(x, W):
    xp = np.zeros((B, 64, 50, 50), np.float32)
    xp[:, :, 1:49, 1:49] = x
    xp = xp.reshape(B, 64, 2500)
    sel = _host_sel()
    wslabs = [_host_weights(W, g) for g in range(4)]
    maps = []
    for core in range(NCORES):
        b, g = core // 4, core % 4
        wmain, wrow, wcol, wcor = wslabs[g]
        maps.append({
            "xp": xp[b],
            "wmain": wmain.reshape(128, 192),
            "wrow": wrow.reshape(128, 128),
            "wcol": wcol.reshape(64, 192),
            "wcor": wcor.reshape(64, 128),
            "sel": sel,
        })
    return maps


def _assemble(results):
    full = np.empty((B, COUT, 96, 96), np.float32)
    for core, res in enumerate(results):
        b, g = core // 4, core % 4
        o = np.asarray(res["out"], np.float32)   # [2px, 32, 6, 384]
        o = o.reshape(2, 2, 16, S, S)            # px, py, i, u, v
        o = o.transpose(2, 3, 1, 4, 0)           # i, u, py, v, px
        full[b, CSH * g:CSH * (g + 1)] = o.reshape(16, 96, 96)
    return full


def kernel(x, W, bias):
    from concourse.bass_utils import run_bass_kernel_spmd

    nc = _build_nc()
    in_maps = _host_inputs(np.asarray(x, np.float32),
                           np.asarray(W, np.float32))
    res = run_bass_kernel_spmd(nc, in_maps, list(range(NCORES)))
    return _assemble(res.results)
